# revision 9
# baseline (speedup 1.0000x reference)
"""Trainium2 Bass kernel for nn_Net_63754494542044 — v2.

Data-parallel over 8 NeuronCores (8 B-samples each).
vs v1: RoIAlign as dense P-matmuls fused in the conv loop (no SWDGE gather,
no feat2 DRAM roundtrip), consolidated weight DMAs, conv2 rhs made
contiguous via x-parity split, GNN mask/degree gating folded into
PSUM-accumulated matmuls (u_ext/Dext trick) with per-state batching.
"""
import sys
sys.path.insert(0, '/opt/trn_rl_repo')
import numpy as np
from contextlib import ExitStack
import concourse.bass as bass
import concourse.tile as tile
from concourse import mybir
from concourse.bass_utils import run_bass_kernel_spmd

# Walrus wait-slot limits (see v1): split multi-wait instructions.
def split_drain_waits(nc, max_waits=1, max_waits_other=1):
    for fn in nc.m.functions:
        for bb in fn.blocks:
            insts = bb.instructions
            i = 0
            while i < len(insts):
                inst = insts[i]
                si = getattr(inst, 'sync_info', None)
                lim = max_waits if isinstance(inst, (mybir.InstDrain, mybir.InstNoOp)) else max_waits_other
                if si is not None and si.on_wait and len(si.on_wait) > lim:
                    waits = list(si.on_wait)
                    keep = waits[-lim:]
                    extra = waits[:-lim]
                    new_nops = []
                    for k in range(0, len(extra), max_waits):
                        chunk = extra[k:k + max_waits]
                        nop = mybir.InstNoOp(
                            name=nc.get_next_instruction_name(),
                            engine=inst.engine,
                        )
                        nop.sync_info = mybir.SyncInfo(on_wait=chunk, on_update=[])
                        nc.register_instruction(nop)
                        new_nops.append(nop)
                    inst.sync_info = mybir.SyncInfo(on_wait=keep, on_update=list(si.on_update))
                    insts[i:i] = new_nops
                    i += len(new_nops)
                i += 1


B, T, N = 64, 4, 6
IMG, CIN = 128, 3
VE, D, P = 64, 256, 4
SCALE = 0.25
NCORE = 8
BC = B // NCORE          # 8 samples per core
NIMG = BC * T            # 32 images per core
NROI = BC * T * N        # 192 rois per core
NROW = BC * N            # 48 gnn rows per core
NPT = NROI * 16          # 3072 sample points per core

IMG_GRP = 4
NGRP = NIMG // IMG_GRP
PIMG = 2 * 33 * 34       # f1 free els per (img): py x (y33, xp2, xj17)
PGRP = 4                 # images per Pmat DMA group


# ---------------- conv1 im2col (host, as v1) ----------------
def conv1_im2col_host(x):  # x [nimg, 3, 128, 128] fp32
    nimg = x.shape[0]
    xp = np.pad(x, ((0, 0), (0, 0), (0, 1), (0, 1)))
    cols = np.empty((45, nimg, 64, 32), np.float32)
    k = 0
    for rt in range(3):
        for ct in range(5):
            for ci in range(3):
                cols[k] = xp[:, ci, rt:rt + 127:2, ct:ct + 125:4]
                k += 1
    return cols  # [45, nimg, 64, 32]


def conv1_weights_host(w_conv1):  # [64, 3, 3, 3] -> [45, 128]
    W2 = np.zeros((45, 128), np.float32)
    for px in range(2):
        for oc in range(64):
            m = px * 64 + oc
            for dy in range(3):
                for dx in range(3):
                    ct = 2 * px + dx
                    for ci in range(3):
                        W2[(dy * 5 + ct) * 3 + ci, m] = w_conv1[oc, ci, dy, dx]
    return W2


def conv2_weights_host(w_conv2):  # [64, 64, 3, 3] -> Wb[dy][g]
    Wb = [[np.zeros((128, 128), np.float32) for _ in range(2)] + [np.zeros((64, 128), np.float32)]
          for _ in range(3)]
    for pxo in range(2):
        for oc in range(64):
            m = pxo * 64 + oc
            for dy in range(3):
                for dx in range(3):
                    x_off = 2 * pxo + dx
                    pxi = x_off % 2
                    Xrel = x_off // 2
                    for ci in range(64):
                        if Xrel < 2:
                            Wb[dy][Xrel][pxi * 64 + ci, m] += w_conv2[oc, ci, dy, dx]
                        else:
                            Wb[dy][2][ci, m] += w_conv2[oc, ci, dy, dx]
    return Wb


def conv1_host(x, w_conv1, b_conv1):
    cols = conv1_im2col_host(x)
    W2 = conv1_weights_host(w_conv1)
    out = np.einsum('kf,kc->cf', cols.reshape(45, -1), W2)
    out = out.reshape(2, 64, -1, 64, 32)
    feat1 = np.empty((x.shape[0], 64, 64, 64), np.float32)
    feat1[..., 0::2] = np.transpose(out[0], (1, 0, 2, 3))
    feat1[..., 1::2] = np.transpose(out[1], (1, 0, 2, 3))
    feat1 += b_conv1[None, :, None, None]
    return feat1


def conv2_host(feat1r, w_conv2, b_conv2):
    nimg = feat1r.shape[0]
    ph = np.zeros((128, nimg, 2, 33, 33), np.float32)
    f = feat1r
    for pxi in range(2):
        for py in range(2):
            ph[pxi * 64:pxi * 64 + 64, :, py, :32, :32] = np.transpose(
                f[:, :, py::2, pxi::2], (1, 0, 2, 3))
    Wb = conv2_weights_host(w_conv2)
    out = np.zeros((128, nimg, 32, 16), np.float32)
    for dy in range(3):
        py, Yoff = dy % 2, dy // 2
        for g in range(3):
            W = Wb[dy][g]
            Ysl = slice(Yoff, Yoff + 32)
            Xidx = g + 2 * np.arange(16)
            rhs = ph[:, :, py, Ysl, :][:, :, :, Xidx]
            if g == 2:
                rhs = rhs[:64]
            out += np.einsum('km,kijx->mijx', W, rhs)
    feat2 = np.empty((nimg, 64, 32, 32), np.float32)
    feat2[..., 0::2] = np.transpose(out[:64], (1, 0, 2, 3))
    feat2[..., 1::2] = np.transpose(out[64:], (1, 0, 2, 3))
    return feat2 + b_conv2[None, :, None, None]


# ---------------- RoIAlign as dense P matrices (host) ----------------
def pool_P_host(rois):
    """rois [NROI, 5] -> P6 [nimg, 2(par), 512(row=(y,j2)), 96(pt=n*16+p)] f32."""
    nroi = rois.shape[0]
    nimg = nroi // N
    W = H = 32
    x1 = rois[:, 1] * SCALE; y1 = rois[:, 2] * SCALE
    x2 = rois[:, 3] * SCALE; y2 = rois[:, 4] * SCALE
    bw = np.maximum(x2 - x1, 1.0) / P
    bh = np.maximum(y2 - y1, 1.0) / P
    grid = np.arange(P, dtype=np.float32) + 0.5
    sx = x1[:, None, None] + bw[:, None, None] * grid[None, None, :]
    sy = y1[:, None, None] + bh[:, None, None] * grid[None, :, None]
    sx = np.broadcast_to(sx, (nroi, P, P)).reshape(-1)
    sy = np.broadcast_to(sy, (nroi, P, P)).reshape(-1)
    x0f = np.clip(np.floor(sx), 0, W - 1)
    y0f = np.clip(np.floor(sy), 0, H - 1)
    lx = np.clip(sx - x0f, 0.0, 1.0).astype(np.float32)
    ly = np.clip(sy - y0f, 0.0, 1.0).astype(np.float32)
    x0 = x0f.astype(np.int32); y0 = y0f.astype(np.int32)
    hi = x0 >= 31
    x0 = np.where(hi, 30, x0); lx = np.where(hi, 1.0, lx)
    hiy = y0 >= 31
    y0 = np.where(hiy, 30, y0); ly = np.where(hiy, 1.0, ly)
    npt = nroi * 16
    img = np.arange(npt) // (N * 16)
    ptl = np.arange(npt) % (N * 16)
    P6 = np.zeros((nimg, 2, 512, 96), np.float32)
    for xx, wx in ((x0, 1.0 - lx), (x0 + 1, lx)):
        par = xx & 1
        j2 = xx >> 1
        for yy, wy in ((y0, 1.0 - ly), (y0 + 1, ly)):
            row = yy * 16 + j2
            np.add.at(P6, (img, par, row, ptl), wx * wy)
    return P6


def pool_host(f2r, rois):
    """Mirror of the device pooling: poolT [64, NPT] (pt-major)."""
    nimg = f2r.shape[0]
    P6 = pool_P_host(rois)
    out = np.zeros((nimg, 64, 96), np.float32)
    for par in range(2):
        rows = np.transpose(f2r[:, :, :, par::2], (0, 2, 3, 1)).reshape(nimg, 512, 64)
        out += np.einsum('irc,irp->icp', rows, P6[:, par])
    return np.transpose(out, (1, 0, 2)).reshape(64, -1)


# ---------------- GNN host mirror (algebra identical to v1) ----------------
def mask_host(coor, r):
    bm = np.zeros((NROW, NROW), np.float32)
    for b in range(BC):
        d = np.linalg.norm(coor[b][:, None, :] - coor[b][None, :, :], axis=-1)
        m = (d <= (r[b][:, None] + r[b][None, :])) & ~np.eye(N, dtype=bool)
        bm[b * N:(b + 1) * N, b * N:(b + 1) * N] = m
    return bm, bm.sum(1)


def internet_host(s, bm, deg, p):
    sw, sb, rw, rb, aw, ab, ow, ob = p
    Wl, Wr = rw[:, :D], rw[:, D:]
    self_d = s @ sw.T + sb
    u = s @ Wl.T + rb
    v = s @ Wr.T
    rel = deg[:, None] * u + bm @ v
    a = np.maximum((self_d + rel) @ aw.T + ab, 0)
    return np.maximum(a @ ow[:, :D].T + s @ ow[:, D:].T + ob, 0)


def gnn_host(obj_t, src_coor, r, inputs):
    states = list(obj_t)
    masks = [mask_host(src_coor[:, t], r) for t in range(4)]
    num_rollouts = int(inputs['num_rollouts'])
    out = []
    for rr in range(num_rollouts):
        cs = []
        for k in range(4):
            p = (inputs['g_self_w'][k], inputs['g_self_b'][k], inputs['g_rel_w'][k],
                 inputs['g_rel_b'][k], inputs['g_aff_w'][k], inputs['g_aff_b'][k],
                 inputs['g_out_w'][k], inputs['g_out_b'][k])
            bm, deg = masks[k]
            cs.append(internet_host(states[k], bm, deg, p))
        s = np.concatenate(cs, -1) @ inputs['agg_w'].T + inputs['agg_b']
        bbox = s @ inputs['dec_w'].T + inputs['dec_b']
        out.append(bbox.reshape(BC, N, 4))
        states = states[1:] + [s]
        coor = bbox[:, 2:].reshape(BC, N, 2)
        masks = masks[1:] + [mask_host(coor, r)]
    return np.stack(out, 1)


def full_host(inputs, shard):
    sl = slice(shard * BC, (shard + 1) * BC)
    x = inputs['x'][sl].reshape(NIMG, CIN, IMG, IMG)
    rois = inputs['rois'][sl].reshape(NROI, 5)
    coor = inputs['src_coor_features'][sl]
    rr5 = rois.reshape(BC, T, N, 5)
    r = (((rr5[..., 4] - rr5[..., 2]) / 2 + (rr5[..., 3] - rr5[..., 1]) / 2) / 2).mean(1)
    f1 = np.maximum(conv1_host(x, inputs['w_conv1'], inputs['b_conv1']), 0)
    f2 = np.maximum(conv2_host(f1, inputs['w_conv2'], inputs['b_conv2']), 0)
    poolT = pool_host(f2, rois)                                 # [64, 3072]
    pool_cp = poolT.T.reshape(NROI, 16, 64)
    Wp = inputs['fc0_w'].reshape(D, 64, 16)
    obj = np.einsum('rpc,dcp->rd', pool_cp, Wp) + inputs['fc0_b']
    obj = np.maximum(obj, 0)
    emb = np.maximum(coor.reshape(NROI, 2) @ inputs['fc0c_w'].T + inputs['fc0c_b'], 0)
    emb = np.maximum(emb @ inputs['fc1c_w'].T + inputs['fc1c_b'], 0)
    o2 = np.maximum(obj @ inputs['red_w'][:, :D].T + emb @ inputs['red_w'][:, D:].T
                    + inputs['red_b'], 0)
    o2 = o2.reshape(BC, T, N, D)
    obj_t = [o2[:, t].reshape(NROW, D) for t in range(4)]
    return gnn_host(obj_t, coor, r, inputs)


# ---------------- packed-weight layout ----------------
def _offsets(spec):
    offs, o = {}, 0
    for name, w in spec:
        offs[name] = o
        o += w
    return offs, o

WCV_SPEC = [('w2a', 384), ('w2b', 384), ('ident', 128)]
WCV_OFF, WCV_N = _offsets(WCV_SPEC)
WB_SPEC = [('fc1ct', 512), ('redoT', 512), ('redeT', 512),
           ('gawT', 2048), ('gowaT', 2048), ('aggT', 2048), ('decT', 8),
           ('wlst', 2048), ('wsdst', 2048), ('wvst', 2048), ('wosst', 2048),
           ('jstat', 48), ('dexts', 48), ('uexts', 1024), ('osexts', 1024)]
WB_OFF, WB_N = _offsets(WB_SPEC)
FB_SPEC = [('b1', 1), ('b2', 1), ('fc0b', 2), ('fc0cb', 2), ('fc1cb', 2),
           ('redb', 2), ('gab', 8), ('aggb', 2)]
FB_OFF, FB_N = _offsets(FB_SPEC)
W64_SPEC = [('w2c', 384), ('fc0t', 4096)]
W64_OFF, W64_N = _offsets(W64_SPEC)
W48_SPEC = [('hm', 192), ('ones48', 128)]
W48_OFF, W48_N = _offsets(W48_SPEC)
W2_SPEC = [('coor', 192), ('fc0ct', 256), ('ones2', 48)]
W2_OFF, W2_N = _offsets(W2_SPEC)


def make_core_inputs(inputs, shard):
    import ml_dtypes
    bf16 = ml_dtypes.bfloat16
    f32 = np.float32
    sl = slice(shard * BC, (shard + 1) * BC)
    x = np.asarray(inputs['x'][sl], f32).reshape(NIMG, CIN, IMG, IMG)
    rois = np.asarray(inputs['rois'][sl], f32).reshape(NROI, 5)
    coor = np.asarray(inputs['src_coor_features'][sl], f32)   # [BC,T,N,2]
    rr5 = rois.reshape(BC, T, N, 5)
    r = (((rr5[..., 4] - rr5[..., 2]) / 2 + (rr5[..., 3] - rr5[..., 1]) / 2) / 2).mean(1)

    d = {}
    cols = conv1_im2col_host(x)                       # [45, NIMG, 64, 32]
    d['im2col45'] = cols.reshape(45, -1).astype(bf16)
    d['w1'] = conv1_weights_host(np.asarray(inputs['w_conv1'])).astype(bf16)

    # Pmat [128, NIMG*768]: cols img*768 + par*384 + chunk*96 + pt
    P6 = pool_P_host(rois)                            # [nimg, 2, 512, 96]
    d['pmat'] = np.ascontiguousarray(
        P6.reshape(NIMG, 2, 4, 128, 96).transpose(3, 0, 1, 2, 4)
    ).reshape(128, NIMG * 768).astype(bf16)

    def t2(w):   # [256, K] -> [K//128, 128, 256] chunks of w.T
        wT = np.ascontiguousarray(np.asarray(w, f32).T)
        K = wT.shape[0]
        return wT.reshape(K // 128, 128, 256)

    def loadg_cols(ws):  # list of 4 [256, 512?]-style t2 stacks -> [128, 4*K/128*256]
        # baseline "loadg" layout: cols = k*(KC*256) + kc*256 + m
        chunks = [t2(w) for w in ws]
        kcn = chunks[0].shape[0]
        out = np.concatenate([c.reshape(kcn * 256, 128).T if False else
                              np.concatenate([c[i] for i in range(kcn)], axis=1)
                              for c in chunks], axis=1)
        return out  # [128, 4*kcn*256]

    Wb = conv2_weights_host(np.asarray(inputs['w_conv2']))
    wcv = np.zeros((128, WCV_N), f32)
    wcv[:, WCV_OFF['w2a']:WCV_OFF['w2a'] + 384] = np.concatenate(
        [Wb[dy][0] for dy in range(3)], 1)
    wcv[:, WCV_OFF['w2b']:WCV_OFF['w2b'] + 384] = np.concatenate(
        [Wb[dy][1] for dy in range(3)], 1)
    wcv[:, WCV_OFF['ident']:WCV_OFF['ident'] + 128] = np.eye(128)
    d['wcv'] = wcv.astype(bf16)
    wb = np.zeros((128, WB_N), f32)
    fc1 = t2(inputs['fc1c_w'])                        # [2, 128, 256]
    wb[:, WB_OFF['fc1ct']:WB_OFF['fc1ct'] + 512] = np.concatenate([fc1[0], fc1[1]], 1)
    redw = np.asarray(inputs['red_w'], f32)
    ro = t2(redw[:, :D]); re = t2(redw[:, D:])
    wb[:, WB_OFF['redoT']:WB_OFF['redoT'] + 512] = np.concatenate([ro[0], ro[1]], 1)
    wb[:, WB_OFF['redeT']:WB_OFF['redeT'] + 512] = np.concatenate([re[0], re[1]], 1)
    wb[:, WB_OFF['gawT']:WB_OFF['gawT'] + 2048] = loadg_cols(
        [inputs['g_aff_w'][k] for k in range(4)])
    gow = np.asarray(inputs['g_out_w'], f32)
    wb[:, WB_OFF['gowaT']:WB_OFF['gowaT'] + 2048] = loadg_cols(
        [gow[k][:, :D] for k in range(4)])
    aggw = np.asarray(inputs['agg_w'], f32)           # [256, 1024]
    at = t2(aggw)                                     # [8, 128, 256]
    wb[:, WB_OFF['aggT']:WB_OFF['aggT'] + 2048] = np.concatenate(list(at), 1)
    decw = np.asarray(inputs['dec_w'], f32)           # [4, 256]
    dT = decw.T.reshape(2, 128, 4)
    wb[:, WB_OFF['decT']:WB_OFF['decT'] + 8] = np.concatenate([dT[0], dT[1]], 1)

    def stackT(ws):  # 4 x [256out, 256in] -> [128, 2*1024]: cols kc*1024 + k*256 + m
        X = np.concatenate([np.asarray(w, f32).T for w in ws], axis=1)   # [256f, 1024]
        return np.concatenate([X[0:128], X[128:256]], axis=1)            # [128, 2048]

    grw = np.asarray(inputs['g_rel_w'], f32)
    wb[:, WB_OFF['wlst']:WB_OFF['wlst'] + 2048] = stackT([grw[k][:, :D] for k in range(4)])
    wb[:, WB_OFF['wsdst']:WB_OFF['wsdst'] + 2048] = stackT(
        [inputs['g_self_w'][k] for k in range(4)])
    wb[:, WB_OFF['wvst']:WB_OFF['wvst'] + 2048] = stackT([grw[k][:, D:] for k in range(4)])
    wb[:, WB_OFF['wosst']:WB_OFF['wosst'] + 2048] = stackT([gow[k][:, D:] for k in range(4)])
    # ext-tile contraction-row layout: 0-47 dyn-A, 48 static, 64-111 dyn-B, 112 static
    js = np.zeros((128, 48), f32)
    js[0:48, 0:48] = np.eye(48); js[48, :] = 1.0
    wb[:, WB_OFF['jstat']:WB_OFF['jstat'] + 48] = js
    dx = np.zeros((128, 48), f32)
    dx[64:112, 0:48] = np.eye(48); dx[112, :] = 1.0
    wb[:, WB_OFF['dexts']:WB_OFF['dexts'] + 48] = dx
    ue = np.zeros((128, 1024), f32)
    ue[48, :] = np.asarray(inputs['g_rel_b'], f32).reshape(-1)    # rb
    ue[112, :] = np.asarray(inputs['g_self_b'], f32).reshape(-1)  # sb
    wb[:, WB_OFF['uexts']:WB_OFF['uexts'] + 1024] = ue
    oe = np.zeros((128, 1024), f32)
    oe[48, :] = np.asarray(inputs['g_out_b'], f32).reshape(-1)    # ob
    wb[:, WB_OFF['osexts']:WB_OFF['osexts'] + 1024] = oe
    d['wb'] = wb.astype(bf16)

    fb = np.zeros((128, FB_N), f32)
    def bcol(b):
        return np.asarray(b, f32).reshape(2, 128).T
    fb[:, FB_OFF['b1']] = np.tile(np.asarray(inputs['b_conv1'], f32), 2)
    fb[:, FB_OFF['b2']] = np.tile(np.asarray(inputs['b_conv2'], f32), 2)
    fb[:, FB_OFF['fc0b']:FB_OFF['fc0b'] + 2] = bcol(inputs['fc0_b'])
    fb[:, FB_OFF['fc0cb']:FB_OFF['fc0cb'] + 2] = bcol(inputs['fc0c_b'])
    fb[:, FB_OFF['fc1cb']:FB_OFF['fc1cb'] + 2] = bcol(inputs['fc1c_b'])
    fb[:, FB_OFF['redb']:FB_OFF['redb'] + 2] = bcol(inputs['red_b'])
    for k in range(4):
        fb[:, FB_OFF['gab'] + 2 * k:FB_OFF['gab'] + 2 * k + 2] = bcol(inputs['g_aff_b'][k])
    fb[:, FB_OFF['aggb']:FB_OFF['aggb'] + 2] = bcol(inputs['agg_b'])
    d['fb'] = fb

    w64 = np.zeros((64, W64_N), f32)
    w64[:, :384] = np.concatenate([Wb[dy][2] for dy in range(3)], 1)
    fc0w = np.asarray(inputs['fc0_w'], f32).reshape(D, 64, 16)   # [d, c, pt]
    fc0t = np.ascontiguousarray(fc0w.transpose(2, 1, 0))         # [pt, c, d]
    w64[:, 384:] = fc0t.transpose(1, 0, 2).reshape(64, 4096)     # [c, pt*256+d]
    d['w64'] = w64.astype(bf16)

    hms = []
    for m in range(4):
        bm, _ = mask_host(coor[:, m], r)
        hms.append(bm)
    w48 = np.zeros((48, W48_N), f32)
    w48[:, :192] = np.concatenate(hms, 1)
    w48[:, 192:320] = 1.0
    d['w48'] = w48.astype(bf16)

    w2 = np.zeros((2, W2_N), f32)
    w2[:, 0:192] = coor.reshape(NROI, 2).T
    w2[:, 192:448] = np.asarray(inputs['fc0c_w'], f32).T
    w2[:, 448:496] = 1.0
    d['w2'] = w2.astype(bf16)

    w4 = np.zeros((4, 2), f32)
    w4[:, 0] = np.asarray(inputs['dec_b'], f32)
    w4[0:2, 1] = np.asarray(inputs['dec_b'], f32)[2:4]
    d['w4'] = w4

    Tmat = np.full((NROW, NROW), -1.0, f32)
    for b in range(BC):
        rs = (r[b][:, None] + r[b][None, :]) ** 2
        np.fill_diagonal(rs, -1.0)
        Tmat[b * N:(b + 1) * N, b * N:(b + 1) * N] = rs
    d['Tm'] = Tmat
    return d


dt = mybir.dt
AF = mybir.ActivationFunctionType
OP = mybir.AluOpType


def build(nc: bass.Bass, stages=('conv', 'roi', 'gnn')):
    f32, bf16, i32 = dt.float32, dt.bfloat16, dt.int32

    def din(name, shape, d):
        return nc.dram_tensor(name, shape, d, kind="ExternalInput")

    im2col = din("im2col45", [45, 65536], bf16)
    w1 = din("w1", [45, 128], bf16)
    wcvd = din("wcv", [128, WCV_N], bf16)
    pmat = din("pmat", [128, NIMG * 768], bf16)
    wbd = din("wb", [128, WB_N], bf16)
    fbd = din("fb", [128, FB_N], f32)
    w64d = din("w64", [64, W64_N], bf16)
    w48d = din("w48", [48, W48_N], bf16)
    w2d = din("w2", [2, W2_N], bf16)
    w4d = din("w4", [4, 2], f32)
    Tmd = din("Tm", [48, 48], f32)

    out = nc.dram_tensor("bbox_out", [8, 8, 6, 4], f32, kind="ExternalOutput")

    with tile.TileContext(nc) as tc, ExitStack() as ctx:
        wp = ctx.enter_context(tc.tile_pool(name="w", bufs=1))
        sp = ctx.enter_context(tc.tile_pool(name="state", bufs=1))

        # conv-critical loads + first input tiles FIRST, then the big packs
        imcp = ctx.enter_context(tc.tile_pool(name="imc", bufs=2))
        pmp = ctx.enter_context(tc.tile_pool(name="pm", bufs=2))
        w1_s = wp.tile([45, 128], bf16, tag="w1")
        nc.sync.dma_start(w1_s[:], w1[:, :])
        fb_s = wp.tile([128, FB_N], f32, tag="fb")
        nc.sync.dma_start(fb_s[:], fbd[:, :])
        wcv_s = wp.tile([128, WCV_N], bf16, tag="wcv")
        nc.sync.dma_start(wcv_s[:], wcvd[:, :])
        w48_s = wp.tile([48, W48_N], bf16, tag="w48")
        nc.sync.dma_start(w48_s[:], w48d[:, :])
        imc0 = imcp.tile([45, IMG_GRP * 2048], bf16, tag="imc")
        nc.sync.dma_start(imc0[:], im2col[:, 0:IMG_GRP * 2048])
        w64_s = wp.tile([64, W64_N], bf16, tag="w64")
        nc.sync.dma_start(w64_s[:], w64d[:, :])
        pm0 = pmp.tile([128, PGRP * 768], bf16, tag="pm")
        nc.sync.dma_start(pm0[:], pmat[:, 0:PGRP * 768])
        w2_s = wp.tile([2, W2_N], bf16, tag="w2")
        nc.sync.dma_start(w2_s[:], w2d[:, :])
        w4_s = wp.tile([4, 2], f32, tag="w4")
        nc.sync.dma_start(w4_s[:], w4d[:, :])
        Tm_s = wp.tile([48, 48], f32, tag="Tm")
        nc.sync.dma_start(Tm_s[:], Tmd[:, :])
        wb_s = wp.tile([128, WB_N], bf16, tag="wb")
        nc.sync.dma_start(wb_s[:], wbd[:, :])

        def wbv(name, lo, n):       # view into packed [128, *] bf16 weights
            o = WB_OFF[name] + lo
            return wb_s[:, o:o + n]

        def fbv(name, lo=0, n=1):
            o = FB_OFF[name] + lo
            return fb_s[:, o:o + n]

        # persistent state
        st = [sp.tile([128, 96], bf16, name=f"st{m}", tag=f"st{m}") for m in range(12)]
        mask_t = [sp.tile([48, 48], bf16, name=f"mask{i}", tag=f"mask{i}") for i in range(4)]
        uext = [sp.tile([128, 1024], bf16, name=f"uext{i}", tag=f"uext{i}") for i in range(4)]
        osext = [sp.tile([128, 1024], bf16, name=f"osext{i}", tag=f"osext{i}") for i in range(4)]
        vrow = [sp.tile([48, 1024], bf16, name=f"vrow{i}", tag=f"vrow{i}") for i in range(4)]
        dext = [sp.tile([128, 48], bf16, name=f"dext{i}", tag=f"dext{i}") for i in range(4)]
        bbox_sb = sp.tile([4, 384], f32, tag="bbox")
        poolT = sp.tile([64, NPT], bf16, tag="poolT")

        def mask_view(m):
            if m < 4:
                return w48_s[0:48, W48_OFF['hm'] + m * 48:W48_OFF['hm'] + (m + 1) * 48]
            return mask_t[m % 4][:]

        ones48_v = w48_s[:, W48_OFF['ones48']:W48_OFF['ones48'] + 128]
        jstat_v = wbv('jstat', 0, 48)
        ident_v = wcv_s[:, WCV_OFF['ident']:WCV_OFF['ident'] + 128]


        emb2 = sp.tile([128, 384], bf16, tag="emb2")

        # ================= conv + pool stage =================
        with ExitStack() as cvx:
          if 'conv' in stages:
            c1ps = cvx.enter_context(tc.tile_pool(name="c1ps", bufs=2, space="PSUM"))
            c2ps = cvx.enter_context(tc.tile_pool(name="c2ps", bufs=2, space="PSUM"))
            tps = cvx.enter_context(tc.tile_pool(name="tps", bufs=1, space="PSUM"))
            pps = cvx.enter_context(tc.tile_pool(name="pps", bufs=1, space="PSUM"))
            f2p = cvx.enter_context(tc.tile_pool(name="f2", bufs=3))

            f1t = [sp.tile([128, IMG_GRP * PIMG], bf16, name=f"f1_{i}", tag=f"f1_{i}")
                   for i in range(2)]
            for f1 in f1t:
                f1v = f1[:].rearrange("p (i py y xp xj) -> p i py y xp xj",
                                      i=IMG_GRP, py=2, y=33, xp=2)
                nc.gpsimd.memset(f1v[:, :, :, 32:33, :, :], 0.0)
                nc.gpsimd.memset(f1v[:, :, :, :, 0:1, 16:17], 0.0)

            evac_i = 0
            def evac(dst, src, relu=False, bias=None):
                nonlocal evac_i
                evac_i += 1
                if evac_i % 2 == 0:
                    if relu:
                        nc.vector.tensor_scalar(out=dst, in0=src, scalar1=bias,
                                                scalar2=0.0, op0=OP.add, op1=OP.max)
                    else:
                        nc.vector.tensor_copy(out=dst, in_=src)
                else:
                    if relu:
                        nc.scalar.activation(out=dst, in_=src, func=AF.Relu, bias=bias)
                    else:
                        nc.scalar.activation(out=dst, in_=src, func=AF.Copy)

            for g in range(NGRP):
                if g == 0:
                    imc = imc0
                else:
                    imc = imcp.tile([45, IMG_GRP * 2048], bf16, tag="imc")
                    gc = IMG_GRP * 2048
                    nc.sync.dma_start(imc[:], im2col[:, g * gc:(g + 1) * gc])
                f1 = f1t[g % 2]
                f1v = f1[:].rearrange("p (i py y xp xj) -> p i py y xp xj",
                                      i=IMG_GRP, py=2, y=33, xp=2)
                for i in range(IMG_GRP):
                    img = g * IMG_GRP + i
                    if img == 0:
                        pm = pm0
                    elif img % PGRP == 0:
                        pm = pmp.tile([128, PGRP * 768], bf16, tag="pm")
                        nc.sync.dma_start(pm[:], pmat[:, img * 768:(img + PGRP) * 768])
                    # conv1: 4 matmuls -> 2 psum [128,1024]
                    pv = []
                    for h in range(2):
                        ps = c1ps.tile([128, 1024], f32, tag="c1")
                        for q in range(2):
                            nc.tensor.matmul(ps[:, q * 512:(q + 1) * 512], lhsT=w1_s[:],
                                             rhs=imc[:, i * 2048 + h * 1024 + q * 512:
                                                     i * 2048 + h * 1024 + (q + 1) * 512],
                                             start=True, stop=True)
                        pv.append(ps)
                    # evac: per (h, xp): dims (py, y16, xj16)
                    for h in range(2):
                        psv = pv[h][:].rearrange("p (y py xj xp) -> p py y xj xp",
                                                 y=16, py=2, xj=16)
                        for xp in range(2):
                            dst = f1v[:, i, :, 16 * h:16 * h + 16, xp, 0:16]
                            evac(dst, psv[:, :, :, :, xp], relu=True, bias=fbv('b1'))
                    # conv2: 9 matmuls -> psum [128, 512]
                    ps2 = c2ps.tile([128, 512], f32, tag="c2")
                    first = True
                    for dy in range(3):
                        py, yo = dy % 2, dy // 2
                        for gsel in range(3):
                            xp, xjo = gsel % 2, gsel // 2
                            rhs = f1v[:, i, py, yo:yo + 32, xp, xjo:xjo + 16]
                            if gsel == 2:
                                rhs = rhs[0:64]
                                lhsT = w64_s[:, W64_OFF['w2c'] + dy * 128:
                                             W64_OFF['w2c'] + (dy + 1) * 128]
                            else:
                                o = WCV_OFF['w2a' if gsel == 0 else 'w2b'] + dy * 128
                                lhsT = wcv_s[:, o:o + 128]
                            nc.tensor.matmul(ps2[:], lhsT=lhsT, rhs=rhs,
                                             start=first, stop=(dy == 2 and gsel == 2))
                            first = False
                    f2sA = f2p.tile([64, 512], bf16, tag="f2sA")
                    f2sB = f2p.tile([64, 512], bf16, tag="f2sB")
                    evac(f2sA[:], ps2[0:64, :], relu=True, bias=fbv('b2')[0:64])
                    evac(f2sB[:], ps2[64:128, :], relu=True, bias=fbv('b2')[64:128])
                    # transposes: rows chunks [128, 64] per (par, b)
                    tp = tps.tile([128, 512], bf16, tag="tp")
                    for par, f2s in ((0, f2sA), (1, f2sB)):
                        for b4 in range(4):
                            nc.tensor.transpose(
                                tp[:, par * 256 + b4 * 64:par * 256 + b4 * 64 + 64],
                                f2s[:, b4 * 128:(b4 + 1) * 128], ident_v[0:64, 0:64])
                    rows = f2p.tile([128, 512], bf16, tag="rows")
                    evac(rows[:], tp[:])
                    # pool matmuls: psum [64, 96]
                    pp = pps.tile([64, 96], f32, tag="pp")
                    nmm = 0
                    for par in range(2):
                        for b4 in range(4):
                            nc.tensor.matmul(
                                pp[:],
                                lhsT=rows[:, par * 256 + b4 * 64:
                                          par * 256 + b4 * 64 + 64],
                                rhs=pm[:, (img % PGRP) * 768 + par * 384 + b4 * 96:
                                        (img % PGRP) * 768 + par * 384 + (b4 + 1) * 96],
                                start=(nmm == 0), stop=(nmm == 7))
                            nmm += 1
                    # poolT pt-major: col = pt*192 + roi  (fc0 rhs becomes contiguous)
                    pdst = poolT[:].rearrange("p (t r) -> p t r", t=16)[
                        :, :, img * 6:(img + 1) * 6]
                    evac(pdst, pp[:].rearrange("p (n t) -> p t n", n=6))

        # ================= fc0 + emb + red =================
        with ExitStack() as gx:
          if 'roi' in stages:
            ops = gx.enter_context(tc.tile_pool(name="ops", bufs=2, space="PSUM"))
            obj = sp.tile([128, 384], bf16, tag="obj")
            for m2 in range(2):
                ps = ops.tile([128, 192], f32, tag="obj")
                for pt_i in range(16):
                    nc.tensor.matmul(
                        ps[:],
                        lhsT=w64_s[:, W64_OFF['fc0t'] + pt_i * 256 + m2 * 128:
                                   W64_OFF['fc0t'] + pt_i * 256 + m2 * 128 + 128],
                        rhs=poolT[:, pt_i * 192:(pt_i + 1) * 192],
                        start=(pt_i == 0), stop=(pt_i == 15))
                nc.scalar.activation(out=obj[:, m2 * 192:(m2 + 1) * 192], in_=ps[:],
                                     func=AF.Relu, bias=fbv('fc0b', m2))
            emb1 = sp.tile([128, 384], bf16, tag="emb1")
            coor_v = w2_s[:, W2_OFF['coor']:W2_OFF['coor'] + 192]
            for m2 in range(2):
                ps = ops.tile([128, 192], f32, tag="emb")
                nc.tensor.matmul(ps[:], lhsT=w2_s[:, W2_OFF['fc0ct'] + m2 * 128:
                                                 W2_OFF['fc0ct'] + (m2 + 1) * 128],
                                 rhs=coor_v, start=True, stop=True)
                nc.scalar.activation(out=emb1[:, m2 * 192:(m2 + 1) * 192], in_=ps[:],
                                     func=AF.Relu, bias=fbv('fc0cb', m2))
            for m2 in range(2):
                ps = ops.tile([128, 192], f32, tag="emb")
                for kc in range(2):
                    nc.tensor.matmul(ps[:], lhsT=wbv('fc1ct', kc * 256 + m2 * 128, 128),
                                     rhs=emb1[:, kc * 192:(kc + 1) * 192],
                                     start=(kc == 0), stop=(kc == 1))
                nc.scalar.activation(out=emb2[:, m2 * 192:(m2 + 1) * 192], in_=ps[:],
                                     func=AF.Relu, bias=fbv('fc1cb', m2))
            o2 = sp.tile([128, 384], bf16, tag="o2")
            for m2 in range(2):
                ps = ops.tile([128, 192], f32, tag="o2")
                for kc in range(2):
                    nc.tensor.matmul(ps[:], lhsT=wbv('redoT', kc * 256 + m2 * 128, 128),
                                     rhs=obj[:, kc * 192:(kc + 1) * 192],
                                     start=(kc == 0), stop=False)
                for kc in range(2):
                    nc.tensor.matmul(ps[:], lhsT=wbv('redeT', kc * 256 + m2 * 128, 128),
                                     rhs=emb2[:, kc * 192:(kc + 1) * 192],
                                     start=False, stop=(kc == 1))
                nc.scalar.activation(out=o2[:, m2 * 192:(m2 + 1) * 192], in_=ps[:],
                                     func=AF.Relu, bias=fbv('redb', m2))
            o2v = o2[:].rearrange("p (m2 b t n) -> p m2 b t n", m2=2, b=8, t=4)
            for m in range(4):
                nc.vector.tensor_copy(
                    out=st[m][:].rearrange("p (m2 b n) -> p m2 b n", m2=2, b=8),
                    in_=o2v[:, :, :, m, :])

        if 'roi' not in stages:
            for m in range(4):
                nc.gpsimd.memset(st[m][:], 0.0)
        if 'gnn' not in stages:
            nc.gpsimd.memset(bbox_sb[:], 0.0)

        # ================= GNN rollouts =================
        with ExitStack() as rx:
          if 'gnn' in stages:
            bps = rx.enter_context(tc.tile_pool(name="bps", bufs=2, space="PSUM"))
            xps = rx.enter_context(tc.tile_pool(name="xps", bufs=2, space="PSUM"))
            ops1 = rx.enter_context(tc.tile_pool(name="ops1", bufs=1, space="PSUM"))
            sps = rx.enter_context(tc.tile_pool(name="sps", bufs=1, space="PSUM"))
            hb = rx.enter_context(tc.tile_pool(name="hbuf", bufs=3))

            ev_i = 0
            def ev2(dst, src, relu=False, bias=None):
                nonlocal ev_i
                ev_i += 1
                if ev_i % 2 == 0:
                    if relu and bias is not None:
                        nc.vector.tensor_scalar(out=dst, in0=src, scalar1=bias,
                                                scalar2=0.0, op0=OP.add, op1=OP.max)
                    elif relu:
                        nc.vector.tensor_scalar(out=dst, in0=src, scalar1=0.0,
                                                scalar2=None, op0=OP.max)
                    else:
                        nc.vector.tensor_copy(out=dst, in_=src)
                else:
                    if relu and bias is not None:
                        nc.scalar.activation(out=dst, in_=src, func=AF.Relu, bias=bias)
                    elif relu:
                        nc.scalar.activation(out=dst, in_=src, func=AF.Relu)
                    else:
                        nc.scalar.activation(out=dst, in_=src, func=AF.Copy)

            def birth(m):
                """u/sd/v/os for state m, batched over the graph slots that
                will actually read it (state m is used by slots k >= m-7)."""
                s = st[m]
                lo_k = max(0, m - 7)
                for wname, dsttile, r0 in (('wlst', uext[m % 4], 0),
                                           ('wvst', vrow[m % 4], 0),
                                           ('wsdst', uext[m % 4], 64),
                                           ('wosst', osext[m % 4], 0)):
                    for h in range(2):
                        ks = max(lo_k, 2 * h)
                        if ks > 2 * h + 1:
                            continue
                        off = (ks - 2 * h) * 256
                        w = 512 - off
                        t = bps.tile([48, 512], f32, tag="b")
                        for kc in range(2):
                            nc.tensor.matmul(t[:, 0:w], lhsT=s[:, kc * 48:kc * 48 + 48],
                                             rhs=wbv(wname, kc * 1024 + h * 512 + off, w),
                                             start=(kc == 0), stop=(kc == 1))
                        ev2(dsttile[r0:r0 + 48, h * 512 + off:(h + 1) * 512], t[:, 0:w])

            def build_dext(m):
                # rows 0-63 <- static selector rows 64-127 (I48 | ones | zeros) * deg
                dd = sps.tile([128, 48], f32, tag="s")
                nc.tensor.matmul(dd[:], lhsT=ones48_v[0:48], rhs=mask_view(m),
                                 start=True, stop=True)
                nc.vector.tensor_tensor(out=dext[m % 4][0:64, :],
                                        in0=dext[m % 4][64:128, :],
                                        in1=dd[0:64, :], op=OP.mult)

            for i in range(4):
                nc.vector.tensor_copy(out=uext[i][:], in_=wbv('uexts', 0, 1024))
                nc.vector.tensor_copy(out=osext[i][:], in_=wbv('osexts', 0, 1024))
                nc.vector.tensor_copy(out=dext[i][:], in_=wbv('dexts', 0, 48))
            for m in range(4):
                build_dext(m)
                birth(m)

            for rr in range(8):
                cs = []
                for k in range(4):
                    m = rr + k
                    x_ps = xps.tile([128, 96], f32, tag="x")
                    for m2 in range(2):
                        nc.tensor.matmul(x_ps[:, m2 * 48:m2 * 48 + 48],
                                         lhsT=uext[m % 4][:, k * 256 + m2 * 128:
                                                          k * 256 + m2 * 128 + 128],
                                         rhs=dext[m % 4][:], start=True, stop=False)
                        nc.tensor.matmul(x_ps[:, m2 * 48:m2 * 48 + 48],
                                         lhsT=vrow[m % 4][:, k * 256 + m2 * 128:
                                                          k * 256 + m2 * 128 + 128],
                                         rhs=mask_view(m), start=False, stop=True)
                    x_sb = hb.tile([128, 96], bf16, tag="x")
                    ev2(x_sb[:], x_ps[:])
                    a_ps = xps.tile([128, 96], f32, tag="a")
                    for m2 in range(2):
                        for kc in range(2):
                            nc.tensor.matmul(a_ps[:, m2 * 48:m2 * 48 + 48],
                                             lhsT=wbv('gawT', k * 512 + kc * 256 + m2 * 128, 128),
                                             rhs=x_sb[:, kc * 48:kc * 48 + 48],
                                             start=(kc == 0), stop=(kc == 1))
                    a_sb = hb.tile([128, 96], bf16, tag="a")
                    for m2 in range(2):
                        ev2(a_sb[:, m2 * 48:m2 * 48 + 48], a_ps[:, m2 * 48:m2 * 48 + 48],
                            relu=True, bias=fbv('gab', 2 * k + m2))
                    o_ps = ops1.tile([128, 96], f32, tag="o")
                    for m2 in range(2):
                        for kc in range(2):
                            nc.tensor.matmul(o_ps[:, m2 * 48:m2 * 48 + 48],
                                             lhsT=wbv('gowaT', k * 512 + kc * 256 + m2 * 128, 128),
                                             rhs=a_sb[:, kc * 48:kc * 48 + 48],
                                             start=(kc == 0), stop=False)
                        nc.tensor.matmul(o_ps[:, m2 * 48:m2 * 48 + 48],
                                         lhsT=osext[m % 4][:, k * 256 + m2 * 128:
                                                           k * 256 + m2 * 128 + 128],
                                         rhs=jstat_v, start=False, stop=True)
                    c_sb = hb.tile([128, 96], bf16, tag=f"cs{k}")
                    ev2(c_sb[:], o_ps[:], relu=True)
                    cs.append(c_sb)
                g_ps = ops1.tile([128, 96], f32, tag="o")
                for m2 in range(2):
                    n = 0
                    for k in range(4):
                        for kc in range(2):
                            nc.tensor.matmul(g_ps[:, m2 * 48:m2 * 48 + 48],
                                             lhsT=wbv('aggT', (k * 2 + kc) * 256 + m2 * 128, 128),
                                             rhs=cs[k][:, kc * 48:kc * 48 + 48],
                                             start=(n == 0), stop=(n == 7))
                            n += 1
                s_new = st[rr + 4]
                for m2 in range(2):
                    nc.vector.tensor_scalar(out=s_new[:, m2 * 48:m2 * 48 + 48],
                                            in0=g_ps[:, m2 * 48:m2 * 48 + 48],
                                            scalar1=fbv('aggb', m2), scalar2=None,
                                            op0=OP.add)
                if rr < 7:
                    m = rr + 4
                    d2_ps = sps.tile([2, 48], f32, tag="s")
                    for kc in range(2):
                        nc.tensor.matmul(d2_ps[:], lhsT=wbv('decT', kc * 4 + 2, 2),
                                         rhs=s_new[:, kc * 48:kc * 48 + 48],
                                         start=(kc == 0), stop=(kc == 1))
                    coorb = hb.tile([2, 48], bf16, tag="coorb")
                    nc.vector.tensor_scalar(out=coorb[:], in0=d2_ps[:],
                                            scalar1=w4_s[0:2, 1:2], scalar2=None, op0=OP.add)
                    cm2 = hb.tile([2, 48], bf16, tag="cm2")
                    nc.vector.tensor_scalar(out=cm2[:], in0=coorb[:], scalar1=-2.0,
                                            scalar2=None, op0=OP.mult)
                    sq = hb.tile([2, 48], bf16, tag="sq")
                    nc.vector.tensor_tensor(out=sq[:], in0=coorb[:], in1=coorb[:], op=OP.mult)
                    ones2_v = w2_s[:, W2_OFF['ones2']:W2_OFF['ones2'] + 48]
                    m_ps = sps.tile([48, 48], f32, tag="s")
                    nc.tensor.matmul(m_ps[:], lhsT=coorb[:], rhs=cm2[:], start=True, stop=False)
                    nc.tensor.matmul(m_ps[:], lhsT=sq[:], rhs=ones2_v, start=False, stop=False)
                    nc.tensor.matmul(m_ps[:], lhsT=ones2_v, rhs=sq[:], start=False, stop=True)
                    nc.vector.tensor_tensor(out=mask_t[m % 4][:], in0=m_ps[:], in1=Tm_s[:],
                                            op=OP.is_le)
                    build_dext(m)
                    birth(m)
                d_ps = sps.tile([4, 48], f32, tag="s")
                for kc in range(2):
                    nc.tensor.matmul(d_ps[:], lhsT=wbv('decT', kc * 4, 4),
                                     rhs=s_new[:, kc * 48:kc * 48 + 48],
                                     start=(kc == 0), stop=(kc == 1))
                bbv = bbox_sb[:].rearrange("f (b q) -> f b q", b=8)[:, :, rr * 6:rr * 6 + 6]
                nc.vector.tensor_scalar(out=bbv, in0=d_ps[:],
                                        scalar1=w4_s[:, 0:1], scalar2=None, op0=OP.add)
        nc.sync.dma_start(
            out[:].rearrange("b rr n f -> f (b rr n)"), bbox_sb[:])
    return nc


_NC = None

def _get_nc():
    global _NC
    if _NC is None:
        nc = bass.Bass()
        build(nc)
        split_drain_waits(nc)
        _NC = nc
    return _NC


def kernel(**inputs):
    nc = _get_nc()
    inputs = {k: np.asarray(v) for k, v in inputs.items()}
    maps = [make_core_inputs(inputs, s) for s in range(NCORE)]
    res = run_bass_kernel_spmd(nc, maps, core_ids=list(range(NCORE)))
    out = np.concatenate([res.results[s]["bbox_out"] for s in range(NCORE)], 0)
    return out.astype(np.float32)
